# revision 10
# baseline (speedup 1.0000x reference)
"""Trainium2 Bass kernel for nn_DSP_33131377721365 (v2).

reference math (x: [4, 32, 720, 720] f32, conv_w: [32, 32, 3, 1] f32):
  s[b,h,w]    = sum_c x[b,c,h,w]
  d[b,h,w]    = (1/9) * sum_{t=0..8} s[b, h+t-4, w+t-4]   (zero padded)
  out[b,o,h,w]= sum_{j=0..2} wsum[o,j] * d[b, h-1+j, w]   (zero padded)
  where wsum[o,j] = sum_c conv_w[o,c,j,0]

Sharding: 8 cores = 4 batches x 2 H-halves (360 rows each); host pre-pads each
shard with 5 halo rows. All HBM I/O in bf16 (host casts) - halves the memory
roofline vs f32.

Per core, 4 H-blocks of 90 output rows, rows on SBUF partitions:
 1. channel sum s: 5 rounds of contiguous-halves tensor_tensor adds on DVE
    (2x bf16 mode; tensor_reduce would be 1x and ~2.1x slower).
 2. 9-tap diagonal pool: 9 accumulating PE matmuls into one PSUM bank; tap t
    uses a shifted-identity band lhsT (h-shift) and a free-dim offset t on the
    rhs AP (w-shift). No SBUF->SBUF shift copies at all.
 3. d evacuated (ScalarE, f32->bf16 cast) into three window tiles for the
    3x1 conv; sub-blocks are (32, 32, 26) output rows so every engine AP has
    a 32-aligned base partition (HW requirement: base % 32 == 0, any size).
 4. 3x1 conv + broadcast to 32 output channels: banded matmuls with 4 output
    channels x 32 rows packed into M=128 (104 for the tail sub-block).
    PSUM evacuated by ScalarE/VectorE (split) with bf16 cast, DMA out on the
    ACT HWDGE ring (input DMAs ride the SP ring).
Host reassembles/casts the bf16 output back to f32.
"""

import numpy as np
import ml_dtypes

import concourse.bass as bass
import concourse.bacc as bacc
import concourse.mybir as mybir
import concourse.tile as tile
from concourse.bass_utils import run_bass_kernel_spmd

FP = mybir.dt.float32
BF = mybir.dt.bfloat16
NPBF = ml_dtypes.bfloat16

B, C, H, W = 4, 32, 720, 720
O = 32
N_CORES = 8
HS = H // 2          # 360 output rows per core
BLK = 90             # output rows per block
NBLK = HS // BLK     # 4
SROWS = BLK + 10     # 100 s-rows per block (pool 8 + conv 2 halo)
M1 = BLK + 2         # 92 d rows per block
SUBNS = (32, 32, 26)  # stage-2 output rows per sub-block (32-aligned windows)
OSUB = 4             # output channels per stage-2 matmul
NOG = O // OSUB      # 8 o-groups
M2MAX = 128
WPAD = 4
SPW = W + 2 * WPAD   # 728
KTAPS = 9
HALO = 5

# stage-2 evacuation engine split: og indices handled by DVE (rest on ACT)
DVE_OGS = (3, 7)


def _build(nc, reps=1):
    xs = nc.declare_dram_parameter("xs", [HS + 2 * HALO, C, W], BF, isOutput=False)
    bd = nc.declare_dram_parameter("bands", [SROWS, KTAPS, NBLK, M1], BF,
                                   isOutput=False)
    ama = nc.declare_dram_parameter("amca", [34, NOG, 128], BF, isOutput=False)
    amb = nc.declare_dram_parameter("amcb", [28, NOG, 104], BF, isOutput=False)
    out = nc.declare_dram_parameter("out", [NBLK * 3, NOG, M2MAX, W], BF,
                                    isOutput=True)

    add = mybir.AluOpType.add

    with tile.TileContext(nc) as tc:
        with (
            tc.tile_pool(name="xa", bufs=2) as xpool,
            tc.tile_pool(name="tr", bufs=1) as tpool,
            tc.tile_pool(name="sp", bufs=2) as spool,
            tc.tile_pool(name="dd", bufs=2) as dpool,
            tc.tile_pool(name="ob", bufs=4) as opool,
            tc.tile_pool(name="cst", bufs=1) as cpool,
            tc.tile_pool(name="ps1", bufs=2, space="PSUM") as ps1pool,
            tc.tile_pool(name="ps2", bufs=3, space="PSUM") as ps2pool,
        ):
            bdt = cpool.tile([SROWS, KTAPS, NBLK, M1], BF)
            nc.sync.dma_start(bdt[:], bd[:])
            amta = cpool.tile([34, NOG, 128], BF)
            nc.sync.dma_start(amta[:], ama[:])
            amtb = cpool.tile([28, NOG, 104], BF)
            nc.sync.dma_start(amtb[:], amb[:])

            for it in range(NBLK * reps):
                blk = it % NBLK
                r0 = blk * BLK

                xa = xpool.tile([SROWS, C, W], BF, tag="xa")
                nc.sync.dma_start(xa[:], xs[r0:r0 + SROWS])

                # channel sum via contiguous-halves tree (bf16 2x DVE mode)
                t1 = tpool.tile([SROWS, 16, W], BF, tag="t1")
                nc.vector.tensor_tensor(out=t1[:], in0=xa[:, 0:16, :],
                                        in1=xa[:, 16:32, :], op=add)
                t2 = tpool.tile([SROWS, 8, W], BF, tag="t2")
                nc.vector.tensor_tensor(out=t2[:], in0=t1[:, 0:8, :],
                                        in1=t1[:, 8:16, :], op=add)
                t3 = tpool.tile([SROWS, 4, W], BF, tag="t3")
                nc.vector.tensor_tensor(out=t3[:], in0=t2[:, 0:4, :],
                                        in1=t2[:, 4:8, :], op=add)
                t4 = tpool.tile([SROWS, 2, W], BF, tag="t4")
                nc.vector.tensor_tensor(out=t4[:], in0=t3[:, 0:2, :],
                                        in1=t3[:, 2:4, :], op=add)
                sp = spool.tile([SROWS, SPW], BF, tag="sp")
                nc.vector.memset(sp[:, 0:WPAD], 0.0)
                nc.vector.memset(sp[:, WPAD + W:SPW], 0.0)
                nc.vector.tensor_tensor(out=sp[:, WPAD:WPAD + W],
                                        in0=t4[:, 0, :], in1=t4[:, 1, :], op=add)

                # 9-tap diagonal pool on PE; evacuate the three conv windows
                # (psum rows [0:34),[32:66),[64:92) - all 32-aligned bases)
                ds0 = dpool.tile([34, W], BF, tag="ds0")
                ds1 = dpool.tile([34, W], BF, tag="ds1")
                ds2 = dpool.tile([28, W], BF, tag="ds2")
                dwin = (ds0, ds1, ds2)
                for w0 in (0, 360):
                    ps1 = ps1pool.tile([M1, 360], FP, tag="ps1")
                    for t in range(KTAPS):
                        nc.tensor.matmul(
                            ps1[:], bdt[:, t, blk, :],
                            sp[:, w0 + t:w0 + t + 360],
                            start=(t == 0), stop=(t == KTAPS - 1),
                        )
                    # engine APs: any size from partition 0, else <=32 parts
                    nc.scalar.copy(out=ds0[0:34, w0:w0 + 360], in_=ps1[0:34, :])
                    nc.scalar.copy(out=ds1[0:32, w0:w0 + 360], in_=ps1[32:64, :])
                    nc.scalar.copy(out=ds1[32:34, w0:w0 + 360], in_=ps1[64:66, :])
                    nc.scalar.copy(out=ds2[0:28, w0:w0 + 360], in_=ps1[64:92, :])

                # 3x1 conv + channel broadcast: banded matmuls, M=4o x ns
                for s in range(3):
                    ns = SUBNS[s]
                    kw, m2 = ns + 2, OSUB * ns
                    amt = amta if s < 2 else amtb
                    dsl = dwin[s]
                    for og in range(NOG):
                        ps2 = ps2pool.tile([M2MAX, W], FP, tag="ps2")
                        lhs = amt[:, og, :]
                        nc.tensor.matmul(ps2[0:m2, 0:512], lhs,
                                         dsl[0:kw, 0:512],
                                         start=True, stop=True)
                        nc.tensor.matmul(ps2[0:m2, 512:W], lhs,
                                         dsl[0:kw, 512:W],
                                         start=True, stop=True)
                        ob = opool.tile([M2MAX, W], BF, tag="ob")
                        if og in DVE_OGS:
                            nc.vector.tensor_copy(out=ob[0:m2, :],
                                                  in_=ps2[0:m2, :])
                        else:
                            nc.scalar.copy(out=ob[0:m2, :], in_=ps2[0:m2, :])
                        nc.scalar.dma_start(out[blk * 3 + s, og, 0:m2],
                                            ob[0:m2, :])
    return nc


def _make_bands(half):
    """[SROWS, 9, NBLK, M1] bf16: tap-t shifted-identity bands, 1/9 scaled.
    Zero the d rows that fall outside the global image (conv zero padding)."""
    bands = np.zeros((SROWS, KTAPS, NBLK, M1), np.float32)
    for t in range(KTAPS):
        for m in range(M1):
            k = m + t
            if k < SROWS:
                bands[k, t, :, m] = 1.0 / KTAPS
    if half == 0:
        bands[:, :, 0, 0] = 0.0        # d row h=-1
    else:
        bands[:, :, NBLK - 1, M1 - 1] = 0.0  # d row h=720
    return bands.astype(NPBF)


def _make_amc(conv_w, ns):
    """[ns+2, NOG, 4*ns] bf16: 3-tap conv bands, 4 o x ns h packed."""
    wsum = conv_w.sum(axis=1)[:, :, 0].astype(np.float64)  # [O, 3]
    amc = np.zeros((ns + 2, NOG, OSUB * ns), np.float32)
    for og in range(NOG):
        for oi in range(OSUB):
            o = og * OSUB + oi
            for m in range(ns):
                for j in range(3):
                    amc[m + j, og, oi * ns + m] = wsum[o, j]
    return amc.astype(NPBF)


def _make_shard(xt_b, h0):
    """xt_b: [H, C, W] bf16 one batch (h-major). Returns padded [HS+10, C, W]."""
    sh = np.zeros((HS + 2 * HALO, C, W), NPBF)
    lo, hi = h0 - HALO, h0 + HS + HALO
    slo, shi = max(lo, 0), min(hi, H)
    sh[slo - lo:shi - lo] = xt_b[slo:shi]
    return sh


def make_in_maps(x, conv_w):
    x = np.ascontiguousarray(np.asarray(x, dtype=np.float32))
    conv_w = np.asarray(conv_w, dtype=np.float32)
    assert x.shape == (B, C, H, W) and conv_w.shape == (O, C, 3, 1)
    xt = np.ascontiguousarray(x.transpose(0, 2, 1, 3)).astype(NPBF)  # [B,H,C,W]
    amca = _make_amc(conv_w, 32)
    amcb = _make_amc(conv_w, 26)
    bands = [_make_bands(0), _make_bands(1)]
    in_maps = []
    for i in range(N_CORES):
        b, half = i // 2, i % 2
        in_maps.append({
            "xs": _make_shard(xt[b], half * HS),
            "bands": bands[half],
            "amca": amca,
            "amcb": amcb,
        })
    return in_maps


def assemble_out(results):
    out = np.empty((B, O, H, W), np.float32)
    for i in range(N_CORES):
        b, half = i // 2, i % 2
        v = np.asarray(results[i]["out"]).astype(np.float32)  # [12,NOG,128,W]
        ov = np.empty((O, HS, W), np.float32)
        for blk in range(NBLK):
            for s in range(3):
                ns = SUBNS[s]
                h0 = blk * BLK + 32 * s
                w = v[blk * 3 + s, :, 0:OSUB * ns, :]
                w = w.reshape(NOG, OSUB, ns, W).reshape(O, ns, W)
                ov[:, h0:h0 + ns, :] = w
        out[b, :, half * HS:(half + 1) * HS, :] = ov
    return out


def kernel(x, conv_w):
    nc = bacc.Bacc("TRN2", target_bir_lowering=False, debug=False,
                   num_devices=N_CORES)
    _build(nc)
    nc.compile()
    res = run_bass_kernel_spmd(nc, make_in_maps(x, conv_w),
                               list(range(N_CORES)), trace=False)
    return assemble_out(res.results)
